# revision 28
# baseline (speedup 1.0000x reference)
"""Multi-head attention (B=4, N=2048, D=1024, H=16) on 8 Trainium2 NeuronCores.

Sharding: core = (batch b = core//2, head-group g = core%2 of 8 heads).
Each core computes qkv + attention for its 8 heads and a *partial* output
projection over its 512 features; the host sums the two partials per batch
and adds the bias (the tensor-parallel unshard).

All matmul operands are bf16 (inputs cast on host). Score matmuls exploit
PE sub-array tiling: per (head-pair p, m-tile g) TWO row-tiled matmuls run
CONCURRENTLY on disjoint halves of the 128x128 array (head 2p contracts on
rows 0-63, head 2p+1 on rows 64-127; K=64 each, no zero-padded q). One
[128,1024] PSUM tile holds S^T for both heads (512 n-cols each), so exp
shape/count is unchanged but the PE spends half the cycles on scores.

attn@v keeps the ones-column-in-v trick (out rows 65) for softmax
denominators; h0/h1 accumulate in parallel po banks (ps_o bufs=2).

Emission is a flat gstep stream (chunk j, pair p, m-tile g) paced for the
ACT engine (exp is the roofline: 33.5M elements at 1/lane/cycle @1.2GHz =
293us). All qkv/projection work runs as EDF-scheduled background thunks:
each kT window/pair-block, v m-tile, and q block carries the gstep index
deadline at which the foreground stream first needs it, so the first exp
fires ~10us in and k/v production for windows 1-3 overlaps chunk 0's exp
stream. Pair epilogues are staged (oT copies at +3, denominator DMAs into
a [33,512] staging tile + ONE batched DVE reciprocal at +5, ones-broadcast
bcp matmuls + normalize multiplies at +10) so the 3.3us reciprocal never
head-of-line-blocks the in-order PE queue.
"""
import heapq
import sys

sys.path.insert(0, '/opt/trn_rl_repo')

import numpy as np
import ml_dtypes

import concourse.bass as bass  # noqa: F401  (registers engines)
import concourse.mybir as mybir
import concourse.tile as tile
from concourse import bacc
from concourse.bass_utils import run_bass_kernel_spmd
dt = mybir.dt

B = 4
N = 2048          # sequence length
D = 1024          # d_model
NH = 16           # total heads
HD = 64           # head dim
NHC = 8           # heads per core
DC = NHC * HD     # 512 features per core
SCALE = HD ** -0.5

P = 128           # partitions
KB = D // P       # 8 k-blocks
NCH = N // 512    # 4 n-chunks of 512
MT = N // P       # 16 m-tiles of 128
DB = DC // P      # 4 head-pair blocks

AV_LAG = 3
PACE = 1          # paced background thunks per gstep beyond deadline-forced


def build_program(debug=False):
    nc = bacc.Bacc("TRN2", target_bir_lowering=False, debug=False,
                   enable_asserts=False, num_devices=8)

    bf16 = dt.bfloat16
    f32 = dt.float32
    f32r = dt.float32r
    Exp = mybir.ActivationFunctionType.Exp
    MULT = mybir.AluOpType.mult

    # host-prearranged layouts: partition-dim first, contiguous fat lines
    xh = nc.dram_tensor("xh", [P, NCH, KB, 512], bf16, kind="ExternalInput")
    wqh = nc.dram_tensor("wqh", [P, KB, DC], bf16, kind="ExternalInput")
    wkh = nc.dram_tensor("wkh", [P, KB, DC], bf16, kind="ExternalInput")
    wvh = nc.dram_tensor("wvh", [P, KB, DC], bf16, kind="ExternalInput")
    wph = nc.dram_tensor("wph", [P, DB, D], bf16, kind="ExternalInput")
    out = nc.dram_tensor("out", [N, D], f32, kind="ExternalOutput")
    if debug:
        dbg_kT = nc.dram_tensor("dbg_kT", [P, DB, N], bf16,
                                kind="ExternalOutput")
        dbg_qT = nc.dram_tensor("dbg_qT", [P, DB, 512], bf16,
                                kind="ExternalOutput")
        dbg_v = nc.dram_tensor("dbg_v", [P, MT, NHC, HD + 1], bf16,
                               kind="ExternalOutput")
        dbg_eS = nc.dram_tensor("dbg_eS", [P, 1024], bf16,
                                kind="ExternalOutput")
        dbg_po = nc.dram_tensor("dbg_po", [HD + 1, 1024], f32,
                                kind="ExternalOutput")
        dbg_at = nc.dram_tensor("dbg_at", [P, DB, 512], bf16,
                                kind="ExternalOutput")

    with tile.TileContext(nc) as tc:
        with tc.tile_pool(name="persist", bufs=1) as persist, \
             tc.tile_pool(name="wq", bufs=1) as wq_pool, \
             tc.tile_pool(name="qTc", bufs=2) as qT_pool, \
             tc.tile_pool(name="xw", bufs=4) as xw_pool, \
             tc.tile_pool(name="ps_S", bufs=2, space="PSUM") as ps_S, \
             tc.tile_pool(name="ps_bg", bufs=2, space="PSUM") as ps_bg, \
             tc.tile_pool(name="ps_o", bufs=2, space="PSUM") as ps_o:

            # ---- persistent SBUF tensors ----
            kT_sb = persist.tile([P, DB, N], bf16, tag="kT")
            # v with a ones column per head: [m-part, m-tile, head, 65]
            v_sb = persist.tile([P, MT, NHC, HD + 1], bf16, tag="v")
            ones_r = persist.tile([P, HD], f32r, tag="ones")
            # denominator staging: pair's two softmax-denominator rows land
            # on partitions 0 and 32; rows 1-31 stay 1.0 so one batched
            # reciprocal covers both heads
            den_sb = persist.tile([33, 512], f32, tag="den")

            wq_sb = wq_pool.tile([P, KB, DC], bf16, tag="wq")
            nc.vector.memset(v_sb[:], 1.0)
            nc.vector.memset(ones_r[:].bitcast(f32), 1.0)
            nc.vector.memset(den_sb[:], 1.0)

            def load_xw(j, label, fine=False):
                xw = xw_pool.tile([P, KB, 512], bf16, tag="xw",
                                  name=f"xw_{label}")
                ap = xh.ap()[:, j, :, :]
                if fine:
                    for kb in range(KB):
                        nc.sync.dma_start(xw[:, kb, :], ap[:, kb, :])
                else:
                    nc.sync.dma_start(xw[:], ap)
                return xw

            # ---- background thunk machinery (EDF) ----
            bg_wait = []          # appended in nondecreasing avail order

            bg_heap = []          # (deadline, seq, thunk)
            bg_seq = [0]

            def bg_schedule(avail, deadline, thunks):
                for t in thunks:
                    bg_wait.append((avail, deadline, bg_seq[0], t))
                    bg_seq[0] += 1

            def bg_tick(idx, next_idx, pace=PACE):
                while bg_wait and bg_wait[0][0] <= idx:
                    _, dl, seq, t = bg_wait.pop(0)
                    heapq.heappush(bg_heap, (dl, seq, t))
                while bg_heap and bg_heap[0][0] <= next_idx:
                    heapq.heappop(bg_heap)[2]()
                for _ in range(pace):
                    if bg_heap:
                        heapq.heappop(bg_heap)[2]()

            def bg_drain_all(idx):
                while bg_wait and bg_wait[0][0] <= idx:
                    _, dl, seq, t = bg_wait.pop(0)
                    heapq.heappush(bg_heap, (dl, seq, t))
                while bg_heap:
                    heapq.heappop(bg_heap)[2]()

            # ---- qkv production thunks ----
            def kT_db_thunks(xw_box, w, db):
                """k projection block (window w, pair db): 4 mm + 1 copy."""
                box = [None]
                thunks = []

                def mm_t(kb0):
                    if kb0 == 0:
                        box[0] = ps_bg.tile([P, 512], f32, tag="bg",
                                            name=f"pk{w}_{db}")
                    for kb in (kb0, kb0 + 1):
                        nc.tensor.matmul(
                            box[0][:],
                            lhsT=wk_box[0][:, kb, db * P:(db + 1) * P],
                            rhs=xw_box[0][:, kb, :],
                            start=(kb == 0), stop=(kb == KB - 1))
                for kb0 in range(0, KB, 2):
                    thunks.append(lambda kb0=kb0: mm_t(kb0))

                def cp_t():
                    nc.vector.tensor_copy(
                        out=kT_sb[:, db, w * 512:(w + 1) * 512],
                        in_=box[0][:])
                thunks.append(cp_t)
                return thunks

            def v_m_thunks(xw_box, w, mc):
                """v for m-tile 4w+mc: 4 mm + 1 copy."""
                m = w * 4 + mc
                box = [None]
                thunks = []

                def mm_t(kb0):
                    if kb0 == 0:
                        box[0] = ps_bg.tile([P, 512], f32, tag="bg",
                                            name=f"pv{m}")
                    for kb in (kb0, kb0 + 1):
                        nc.tensor.matmul(
                            box[0][:],
                            lhsT=xw_box[0][:, kb, mc * P:(mc + 1) * P],
                            rhs=wv_box[0][:, kb, :],
                            start=(kb == 0), stop=(kb == KB - 1))
                for kb0 in range(0, KB, 2):
                    thunks.append(lambda kb0=kb0: mm_t(kb0))

                def cp_t():
                    nc.vector.tensor_copy(
                        out=v_sb[:, m, :, 0:HD],
                        in_=box[0][:].rearrange("p (h d) -> p h d", h=NHC))
                thunks.append(cp_t)
                return thunks

            qT_tiles = [None] * NCH

            def q_db_thunks(jn, xw_box, qt, db):
                """q block db of chunk jn: 4 mm + 1 copy (no zero pad)."""
                box = [None]
                thunks = []

                def mm_t(kb0):
                    if kb0 == 0:
                        box[0] = ps_bg.tile([P, 512], f32, tag="bg",
                                            name=f"pq{jn}_{db}")
                    for kb in (kb0, kb0 + 1):
                        nc.tensor.matmul(
                            box[0][:],
                            lhsT=wq_sb[:, kb, db * P:(db + 1) * P],
                            rhs=xw_box[0][:, kb, :],
                            start=(kb == 0), stop=(kb == KB - 1))
                for kb0 in range(0, KB, 2):
                    thunks.append(lambda kb0=kb0: mm_t(kb0))

                def cp_t():
                    nc.vector.tensor_copy(out=qt[:, db, :], in_=box[0][:])
                thunks.append(cp_t)
                return thunks

            def schedule_qT(jn, avail, deadline):
                qt = qT_pool.tile([P, DB, 512], bf16, tag="qTc",
                                  name=f"qT{jn}")
                qT_tiles[jn] = qt
                thunks = []
                for db in range(DB):
                    thunks.extend(q_db_thunks(jn, xw_boxes[jn], qt, db))
                bg_schedule(avail, deadline, thunks)

            def emit_proj_thunks(j):
                """Projection of chunk j (at_j is bf16)."""
                at_j = at_tiles[j]
                thunks = []
                box = [None]
                for ns in range(4):
                    for ec in range(2):
                        def mm_t(ns, ec, cb0):
                            if cb0 == 0:
                                box[0] = ps_bg.tile([P, 512], f32, tag="bg",
                                                    name=f"pp{ns}_{ec}")
                            for cb in (cb0, cb0 + 1):
                                nc.tensor.matmul(
                                    box[0][:],
                                    lhsT=at_j[:, cb, ns * P:(ns + 1) * P],
                                    rhs=wp_box[0][:, cb,
                                                  ec * 512:(ec + 1) * 512],
                                    start=(cb == 0), stop=(cb == DB - 1))
                        for cb0 in range(0, DB, 2):
                            thunks.append(
                                lambda ns=ns, ec=ec, cb0=cb0: mm_t(ns, ec, cb0))
                        def cp_t(ns=ns, ec=ec):
                            osb = out_pool.tile([P, 512], f32, tag="osb",
                                                name=f"osb{ns}_{ec}")
                            nc.vector.tensor_copy(out=osb[:], in_=box[0][:])
                            nc.sync.dma_start(
                                out.ap()[j * 512 + ns * P:
                                         j * 512 + (ns + 1) * P,
                                         ec * 512:(ec + 1) * 512],
                                osb[:])
                        thunks.append(cp_t)
                return thunks

            wk_box = [None]
            wv_box = [None]
            wp_box = [None]
            xw_boxes = [[None] for _ in range(NCH)]

            # ---- prelude: DMAs + minimal serial work for gstep 0 ----
            wkv_scope = tc.tile_pool(name="wkv", bufs=1)
            wkv_pool = wkv_scope.__enter__()
            wk_sb = wkv_pool.tile([P, KB, DC], bf16, tag="wk")
            wv_sb = wkv_pool.tile([P, KB, DC], bf16, tag="wv")
            wk_box[0] = wk_sb
            wv_box[0] = wv_sb
            xw0 = xw_pool.tile([P, KB, 512], bf16, tag="xw", name="xw_kv0")
            xw0_ap = xh.ap()[:, 0, :, :]
            # first k-blocks land first so kT(w0,db0)/q0(db0) start early
            for kb in range(KB):
                nc.sync.dma_start(wk_sb[:, kb, :], wkh.ap()[:, kb, :])
                nc.sync.dma_start(xw0[:, kb, :], xw0_ap[:, kb, :])
                nc.sync.dma_start(wq_sb[:, kb, :], wqh.ap()[:, kb, :])
            xw_boxes[0][0] = xw0

            # serial: kT(w0, db0) and q0(db0) unblock gstep 0
            for t in kT_db_thunks(xw_boxes[0], 0, 0):
                t()
            qt0 = qT_pool.tile([P, DB, 512], bf16, tag="qTc", name="qT0")
            qT_tiles[0] = qt0
            for t in q_db_thunks(0, xw_boxes[0], qt0, 0):
                t()
            # prefetches queue behind the critical-path DMAs above
            nc.sync.dma_start(wv_sb[:], wvh.ap())
            for w in range(1, NCH):
                xw_boxes[w][0] = load_xw(w, f"kv{w}")
            # everything else is deadline-scheduled: kT(w,db) first needed
            # by S(p=db, g=4w) at gstep 16db+4w; v(m) by av at gstep m+3;
            # q0(db) by S(p=db) at 16db
            for w in range(NCH):
                for db in range(DB):
                    if (w, db) == (0, 0):
                        continue
                    bg_schedule(0, 16 * db + 4 * w,
                                kT_db_thunks(xw_boxes[w], w, db))
                for mc in range(4):
                    bg_schedule(0, 4 * w + mc + 2,
                                v_m_thunks(xw_boxes[w], w, mc))
            for db in range(1, DB):
                bg_schedule(0, 16 * db, q_db_thunks(0, xw_boxes[0], qt0, db))

            expS_scope = tc.tile_pool(name="expS", bufs=5)
            expS_pool = expS_scope.__enter__()
            at_scope = tc.tile_pool(name="at", bufs=2)
            at_pool = at_scope.__enter__()
            small_scope = tc.tile_pool(name="small", bufs=1)
            small_pool = small_scope.__enter__()
            out_scope = tc.tile_pool(name="outsb", bufs=2)
            out_pool = out_scope.__enter__()

            # ---- flat gstep stream: (chunk j, head-pair p, m-tile g) ----
            gsteps = [(j, p, g) for j in range(NCH) for p in range(DB)
                      for g in range(MT)]
            CHUNK = DB * MT
            at_tiles = [None] * NCH
            eS_q = {}
            po_pairs = {}
            pending_p1 = []
            pending_rcp = []
            pending_ep = []

            def emit_S(j, p, g):
                S = ps_S.tile([P, 1024], f32, tag="S", name=f"S{p}_{g}")
                qt = qT_tiles[j]
                nc.tensor.matmul(S[:, 0:512],
                                 lhsT=kT_sb[0:HD, p, g * P:(g + 1) * P],
                                 rhs=qt[0:HD, p, :],
                                 start=True, stop=True)
                nc.tensor.matmul(S[:, 512:1024],
                                 lhsT=kT_sb[HD:P, p, g * P:(g + 1) * P],
                                 rhs=qt[HD:P, p, :],
                                 start=True, stop=True)
                return S

            def emit_av(idx2):
                j, p, g = gsteps[idx2]
                eSp = eS_q.pop((j, p, g))
                key = (j, p)
                if key not in po_pairs:
                    po_pairs[key] = [
                        ps_o.tile([P, 512], f32, tag="o",
                                  name=f"po{j}_{p}_{h}")
                        for h in range(2)]
                po0, po1 = po_pairs[key]
                nc.tensor.matmul(po0[0:HD + 1, :],
                                 lhsT=v_sb[:, g, 2 * p, :],
                                 rhs=eSp[:, 0:512],
                                 start=(g == 0), stop=(g == MT - 1))
                nc.tensor.matmul(po1[0:HD + 1, :],
                                 lhsT=v_sb[:, g, 2 * p + 1, :],
                                 rhs=eSp[:, 512:1024],
                                 start=(g == 0), stop=(g == MT - 1))
                if g == MT - 1:
                    pending_p1.append([2, po0, po1, j, p])

            def emit_epilogue_p1(po0, po1, j, p):
                """Free the po banks; stage denominator rows via DMA."""
                oT0 = small_pool.tile([HD + 1, 512], f32, tag="oT0",
                                      name="oT0")
                oT1 = small_pool.tile([HD + 1, 512], f32, tag="oT1",
                                      name="oT1")
                nc.vector.tensor_copy(out=oT0[:], in_=po0[0:HD + 1, :])
                nc.vector.tensor_copy(out=oT1[:], in_=po1[0:HD + 1, :])
                nc.sync.dma_start(den_sb[0:1, :], oT0[HD:HD + 1, :])
                nc.sync.dma_start(den_sb[32:33, :], oT1[HD:HD + 1, :])
                if debug and (j, p) == (0, 0):
                    nc.sync.dma_start(dbg_po.ap()[:, 0:512], oT0[:])
                    nc.sync.dma_start(dbg_po.ap()[:, 512:1024], oT1[:])
                return oT0, oT1

            def emit_epilogue_recip():
                rcp = small_pool.tile([33, 512], f32r, tag="rcp", name="rcp")
                with nc.allow_low_precision(reason="softmax recip to f32r"):
                    nc.vector.reciprocal(rcp[:], den_sb[:])
                return rcp

            def emit_epilogue_p2(j, p, oT0, oT1, rcp):
                at = at_tiles[j]
                with nc.allow_low_precision(reason="softmax normalize bf16"):
                    bcp0 = ps_bg.tile([P, 512], f32, tag="bg", name="bcp0")
                    nc.tensor.matmul(bcp0[0:HD, :],
                                     lhsT=ones_r[0:1, :],
                                     rhs=rcp[0:1, :],
                                     start=True, stop=True)
                    nc.vector.tensor_tensor(
                        out=at[0:HD, p, :], in0=oT0[0:HD, :],
                        in1=bcp0[0:HD, :], op=MULT)
                    bcp1 = ps_bg.tile([P, 512], f32, tag="bg", name="bcp1")
                    nc.tensor.matmul(bcp1[0:HD, :],
                                     lhsT=ones_r[32:33, :],
                                     rhs=rcp[32:33, :],
                                     start=True, stop=True)
                    tmp1 = small_pool.tile([HD, 512], bf16, tag="tmp1",
                                           name="tmp1")
                    nc.vector.tensor_tensor(
                        out=tmp1[:], in0=oT1[0:HD, :],
                        in1=bcp1[0:HD, :], op=MULT)
                    nc.sync.dma_start(at[HD:P, p, :], tmp1[:])

            def drain_queues():
                # later stages first: den_sb is shared, so pair n's recip
                # must be emitted before pair n+1's den DMAs
                for ep in pending_ep:
                    ep[0] -= 1
                while pending_ep and pending_ep[0][0] <= 0:
                    _, j, p, oT0, oT1, rcp = pending_ep.pop(0)
                    emit_epilogue_p2(j, p, oT0, oT1, rcp)
                for ep in pending_rcp:
                    ep[0] -= 1
                while pending_rcp and pending_rcp[0][0] <= 0:
                    _, j, p, oT0, oT1 = pending_rcp.pop(0)
                    rcp = emit_epilogue_recip()
                    # bcp matmuls enter the in-order PE queue only after
                    # the 3.3us batched DVE reciprocal has surely finished
                    pending_ep.append([6, j, p, oT0, oT1, rcp])
                for ep in pending_p1:
                    ep[0] -= 1
                while pending_p1 and pending_p1[0][0] <= 0:
                    _, po0, po1, j, p = pending_p1.pop(0)
                    oT0, oT1 = emit_epilogue_p1(po0, po1, j, p)
                    pending_rcp.append([1, j, p, oT0, oT1])

            def start_chunk(c, idx):
                if c == 0:
                    wp_scope = tc.tile_pool(name="wp", bufs=1)
                    wp_pool = wp_scope.__enter__()
                    wp_box.append(wp_scope)  # keep scope alive
                    wp_sb = wp_pool.tile([P, DB, D], bf16, tag="wp")
                    nc.sync.dma_start(wp_sb[:], wph.ap())
                    wp_box[0] = wp_sb
                if debug and c == 1:
                    nc.sync.dma_start(dbg_kT.ap(), kT_sb[:])
                    nc.sync.dma_start(dbg_v.ap(), v_sb[:])
                    nc.sync.dma_start(dbg_qT.ap(), qT_tiles[0][:])
                if debug and c == 2:
                    nc.sync.dma_start(dbg_at.ap(), at_tiles[0][:])
                at_tiles[c] = at_pool.tile([P, DB, 512], bf16, tag="at",
                                           name=f"at{c}")
                if c + 1 < NCH:
                    schedule_qT(c + 1, idx, idx + CHUNK)
                if c >= 1:
                    bg_schedule(idx + 16, idx + CHUNK,
                                emit_proj_thunks(c - 1))

            start_chunk(0, 0)
            S_next = emit_S(*gsteps[0])
            for idx, (j, p, g) in enumerate(gsteps):
                S_cur = S_next
                eS = expS_pool.tile([P, 1024], bf16, tag="e",
                                    name=f"eS{p}_{g}")
                nc.scalar.activation(eS[:], S_cur[:], Exp, scale=SCALE)
                if debug and idx == 0:
                    nc.sync.dma_start(dbg_eS.ap(), eS[:])
                eS_q[(j, p, g)] = eS
                if idx >= AV_LAG:
                    emit_av(idx - AV_LAG)
                drain_queues()
                if idx + 1 < len(gsteps):
                    if (idx + 1) % CHUNK == 0:
                        start_chunk((idx + 1) // CHUNK, idx + 1)
                    bg_tick(idx, idx + 1)
                    S_next = emit_S(*gsteps[idx + 1])

            for idx2 in range(len(gsteps) - AV_LAG, len(gsteps)):
                emit_av(idx2)
            while pending_p1 or pending_rcp or pending_ep:
                drain_queues()
            bg_drain_all(10 ** 9)

            # final chunk's projection
            for t in emit_proj_thunks(NCH - 1):
                t()

            if len(wp_box) > 1:
                wp_box[1].__exit__(None, None, None)
            out_scope.__exit__(None, None, None)
            small_scope.__exit__(None, None, None)
            at_scope.__exit__(None, None, None)
            expS_scope.__exit__(None, None, None)
            wkv_scope.__exit__(None, None, None)

    nc.compile()
    return nc


_CACHE: dict = {}


def _get_program():
    if "nc" not in _CACHE:
        _CACHE["nc"] = build_program()
    return _CACHE["nc"]


def make_in_maps(x, w_qkv, w_proj):
    """Host-side sharding: per-core input dict (bf16)."""
    bf = ml_dtypes.bfloat16
    x = np.asarray(x, dtype=np.float32)
    w_qkv = np.asarray(w_qkv, dtype=np.float32)
    w_proj = np.asarray(w_proj, dtype=np.float32)
    in_maps = []
    for core in range(8):
        b, g = divmod(core, 2)
        gsl = slice(g * DC, (g + 1) * DC)
        xT = x[b].T.astype(bf)                                    # [D, N]
        wqT = w_qkv[0 * D:1 * D][gsl].T.astype(bf)                # [D, DC]
        wkT = w_qkv[1 * D:2 * D][gsl].T.astype(bf)
        wvT = w_qkv[2 * D:3 * D][gsl].T.astype(bf)
        wpT = w_proj[:, gsl].T.astype(bf)                         # [DC, D]
        in_maps.append({
            # [P, NCH, KB, 512]: contiguous 8KB lines per window
            "xh": np.ascontiguousarray(
                xT.reshape(KB, P, NCH, 512).transpose(1, 2, 0, 3)),
            "wqh": np.ascontiguousarray(
                wqT.reshape(KB, P, DC).transpose(1, 0, 2)),
            "wkh": np.ascontiguousarray(
                wkT.reshape(KB, P, DC).transpose(1, 0, 2)),
            "wvh": np.ascontiguousarray(
                wvT.reshape(KB, P, DC).transpose(1, 0, 2)),
            "wph": np.ascontiguousarray(
                wpT.reshape(DB, P, D).transpose(1, 0, 2)),
        })
    return in_maps


def run(x, w_qkv, w_proj, b_proj, **spmd_kwargs):
    nc = _get_program()
    in_maps = make_in_maps(x, w_qkv, w_proj)
    try:
        res = run_bass_kernel_spmd(nc, in_maps, list(range(8)),
                                   **spmd_kwargs)
    except Exception:
        # device occasionally comes up wedged on the first attempt
        res = run_bass_kernel_spmd(nc, in_maps, list(range(8)),
                                   **spmd_kwargs)
    b_proj = np.asarray(b_proj, dtype=np.float32)
    outp = np.empty((B, N, D), dtype=np.float32)
    for b in range(B):
        outp[b] = (res.results[2 * b]["out"] + res.results[2 * b + 1]["out"]
                   + b_proj[None, :])
    return outp, res


def kernel(x, w_qkv, w_proj, b_proj):
    outp, _ = run(x, w_qkv, w_proj, b_proj)
    return outp
